# revision 36
# baseline (speedup 1.0000x reference)
"""Trainium2 Bass kernel for blockwise-DCT + high-freq mask (nn_DCT_46119358825006).

Math (reference):
  X = floor(255 * x)                        # [4096, 4096]
  out[8hb+m, 8k+j] = sum_i Db[m,i] * sum_wb Dw[k,wb] * X[8hb+i, 8wb+j]
  masked to zero for m < 2 or j < 2.
  Db = 8-point orthonormal DCT-II, Dw = 512-point orthonormal DCT-II.

Sharding: data-parallel over rows. 8 cores x 512 rows each, zero comm.

Approximations (rel-err gate is 2e-2; this lands ~6e-3):
  - xb = bf16-TRUNCATE(x) with the 255x scale folded into dbm2, skipping
    the exact floor. bf16-truncation is the high 16 bits of the fp32, so
    the integerize is a native bf16->bf16 strided copy of the bitcast
    input (no ALU op). Error: frac-of-255x mean lands in masked DC
    coords; truncation is a ~2^-9 multiplicative signal error; zero-mean
    noise ~0.4% relative l2 total.
  - output shipped as bf16 (~0.2% rel), host-upcast to fp32.

Per-core dataflow (512 rows x 4096 cols), per 128-row chunk rc:
  in-DMA      xin [128, 4096] f32 (2 chunked DMAs on SP)
  integerize  xb[:, ws, j'] = high-half bf16 view of xin, 6 live j only
              (one bitcast copy per wc, DVE/Act alternating)
  psB         row-DCT fused with transpose: pb[w, (m',g)] via
              dbm2 = kron(I16, Db[2:8,:].T); w lands on partitions
  psC         column DCT with DwT 128x128 chunks STATIONARY and the
              row-DCT'd data moving: out partitions = k (all 128 used),
              accumulate over 4 w-chunks per (kc, j-half)
  out         oc [128 k, 4 kc, 6 j', 96 n] bf16, ONE DMA per rc (on Act);
              only live coefficients ship (56% x 50% of baseline bytes).
              Host scatters into the zero-filled full fp32 output.

Measured on HW (in-NEFF loop slope, 8 cores concurrent): ~40 us/iter vs
58.5 us baseline. The binding constraint is SBUF port bandwidth (~1.05
TB/s/core observed): compute-only runs at 27.7 us, in-DMA+compute at 37.5
(the in-stream contends ~10 us even with zero data dependency), +out-DMA
~40. The pure in+out DMA floor is 34.3 us (chip HBM ~2.5 TB/s across 8
cores), so per-iteration time tracks total SBUF bytes (10.5 MB/rc).

Variants measured SLOWER and reverted (paired interleaved A/B):
  - psC merged copies via [P,2,512] bank-aligned halves: +2.3us (copy
    waits both 4-matmul chains; psC pool parallelism halved)
  - 576-wide psC matmul (1 chain/kc): walrus ISA check rejects matmul
    output free size > 512 fp32
  - ut pool bufs 2->3: exactly neutral (dependency slack is not binding)
  - skewed psB(rc)/psC(rc-1) emission, split out-DMA, 4-chunk in-DMA,
    3:2 DVE:Act split, j-major xb, fp32-direct psB: all slower (see git
    of prior session notes in memory).
"""

import numpy as np
import ml_dtypes

BLOCK = 8
H = W = 4096
Wb = W // BLOCK          # 512
N_CORES = 8
R = H // N_CORES         # 512 rows per core
P = 128                  # partitions
NRC = R // P             # 4 row-chunks per core
NWC = Wb // P            # 4 w-chunks
NKC = Wb // P            # 4 k-chunks
NJ = 6                   # live j values (j = 2..7)
NL = P - 2 * (P // BLOCK)   # 96 live rows per 128-row chunk (16 groups x 6 m')


def _dct_mat(N):
    n = np.arange(N, dtype=np.float64)
    k = n[:, None]
    D = np.cos(np.pi * (2.0 * n[None, :] + 1.0) * k / (2.0 * N))
    scale = np.where(np.arange(N) == 0, np.sqrt(1.0 / N), np.sqrt(2.0 / N))
    return D * scale[:, None]


def make_consts():
    bf16 = ml_dtypes.bfloat16
    DwT = np.ascontiguousarray(_dct_mat(Wb).T)    # [w, k]
    Db = _dct_mat(BLOCK)
    # 255x scale folded into the row-DCT matrix: the "integerize" is then a
    # pure bf16 copy (the high half of an fp32 IS its bf16 truncation)
    dbm2 = 255.0 * np.kron(np.eye(P // BLOCK), Db[2:BLOCK, :].T)   # [128, 96]
    return {
        "dwt_b": np.ascontiguousarray(DwT.reshape(NWC, P, Wb)).astype(bf16),
        "dbm2": np.ascontiguousarray(dbm2).astype(bf16),
    }


def build_nc(n_loop=1):
    import contextlib
    import concourse.mybir as mybir
    import concourse.tile as tile
    from concourse import bacc

    f32 = mybir.dt.float32
    bf16 = mybir.dt.bfloat16

    nc = bacc.Bacc("TRN2", target_bir_lowering=False, debug=False,
                   num_devices=N_CORES)

    x_dram = nc.dram_tensor("x", [R, W], f32, kind="ExternalInput").ap()
    dwt_dram = nc.dram_tensor("dwt_b", [NWC, P, Wb], bf16,
                              kind="ExternalInput").ap()
    dbm2_dram = nc.dram_tensor("dbm2", [P, NL], bf16,
                               kind="ExternalInput").ap()
    # compact live-coefficient output: [rc, k_local, kc, j', (g, m')] bf16
    out_dram = nc.dram_tensor("out", [NRC, P, NKC, NJ, NL], bf16,
                              kind="ExternalOutput").ap()

    with tile.TileContext(nc) as tc:
        with (
            tc.tile_pool(name="consts", bufs=1) as consts,
            tc.tile_pool(name="xin", bufs=3) as xinp,
            tc.tile_pool(name="xb", bufs=2) as xbp,
            tc.tile_pool(name="ut", bufs=2) as utp,
            tc.tile_pool(name="oc", bufs=3) as ocp,
            tc.tile_pool(name="psB", bufs=2, space="PSUM") as psB,
            tc.tile_pool(name="psC", bufs=4, space="PSUM") as psC,
        ):
            dwts = []
            for wc in range(NWC):
                t = consts.tile([P, Wb], bf16, name=f"dw{wc}", tag=f"dw{wc}")
                nc.sync.dma_start(t, dwt_dram[wc])
                dwts.append(t)
            dbm2 = consts.tile([P, NL], bf16)
            nc.sync.dma_start(dbm2, dbm2_dram)

            if n_loop < 0:      # unrolled python loop (TimelineSim-friendly)
                for _ in range(-n_loop):
                    _emit_body(nc, mybir,
                               pools=(xinp, xbp, utp, ocp, psB, psC),
                               cb=(dwts, dbm2),
                               drams=(x_dram, out_dram))
            else:
                loop_ctx = (tc.For_i(0, n_loop, 1) if n_loop > 1
                            else contextlib.nullcontext())
                with loop_ctx:
                    _emit_body(nc, mybir,
                               pools=(xinp, xbp, utp, ocp, psB, psC),
                               cb=(dwts, dbm2),
                               drams=(x_dram, out_dram))

    nc.compile()
    return nc


MODE = "full"   # "full" | "dma" (DMA floor) | "noio" (compute only) | "noout"
SKEW = False    # skewed emission measured no faster, and noskew is simpler


def _emit_body(nc, mybir, pools, cb, drams):
    f32 = mybir.dt.float32
    bf16 = mybir.dt.bfloat16
    xinp, xbp, utp, ocp, psB, psC = pools
    dwts, dbm2 = cb
    x_dram, out_dram = drams

    CW = W // 2          # 2048 input cols per in-DMA chunk

    if MODE == "dma":
        for rc in range(NRC):
            xin = xinp.tile([P, W], f32, name=f"xin{rc}", tag="xin")
            for h in range(2):
                nc.sync.dma_start(xin[:, h * CW:(h + 1) * CW],
                                  x_dram[rc * P:(rc + 1) * P,
                                         h * CW:(h + 1) * CW])
            oc = ocp.tile([P, NKC, NJ, NL], bf16, name=f"oc{rc}", tag="oc")
            nc.vector.tensor_scalar(oc[:, 0, 0, 0:1], xin[:, 0:1], 0.0, None,
                                    op0=mybir.AluOpType.mult)
            nc.scalar.dma_start(out_dram[rc], oc)
        return

    # vector-ish ops split 3:2 DVE:Act (DVE is faster; Pool has no PSUM access
    # and gpsimd elementwise is a catastrophically slow software Q7 path)
    PAT = (0, 1)             # 0 = DVE, 1 = Act

    def ve_scale(n, d, s):
        if PAT[n % 2] == 0:
            nc.vector.tensor_scalar(d, s, 255.0, None,
                                    op0=mybir.AluOpType.mult)
        else:
            nc.scalar.activation(d, s, mybir.ActivationFunctionType.Copy,
                                 scale=255.0)

    def ve_copy(n, d, s):
        if PAT[n % 2] == 0:
            nc.vector.tensor_copy(d, s)
        else:
            nc.scalar.copy(d, s)

    nv = [0]

    def front(rc):
        xin = xinp.tile([P, W], f32, name=f"xin{rc}", tag="xin")
        if MODE == "noio":
            nc.gpsimd.memset(xin[:, 0:1], 0.0)   # satisfy tile allocation
        elif MODE == "indep":
            # same DMA traffic, but into a scratch tile nothing reads:
            # isolates dependency stalls from raw bandwidth contention
            nc.gpsimd.memset(xin[:, 0:1], 0.0)
            scr = xinp.tile([P, W], f32, name=f"scr{rc}", tag="scr")
            for h in range(2):
                nc.sync.dma_start(scr[:, h * CW:(h + 1) * CW],
                                  x_dram[rc * P:(rc + 1) * P,
                                         h * CW:(h + 1) * CW])
        else:
            for h in range(2):
                nc.sync.dma_start(xin[:, h * CW:(h + 1) * CW],
                                  x_dram[rc * P:(rc + 1) * P,
                                         h * CW:(h + 1) * CW])
        # bf16(x) by truncation = the high 16 bits of each fp32 (little-
        # endian: odd bf16 index). The "integerize" is then a native
        # bf16->bf16 copy (16-bit DVE path), with the 255x scale folded
        # into dbm2. Truncation-vs-RNE is a ~2^-9 multiplicative error on
        # the signal, ~0.2% in the output.
        xin_h = xin.bitcast(bf16).rearrange("p (w j h) -> p w j h",
                                            j=BLOCK, h=2)
        # compact bf16 X: only the 6 live j columns, layout [p, w, j'].
        # KEEP j interleaved: the copy then reads 12-of-16B runs and writes
        # dense; a j-major layout needs a scattered-write transpose,
        # measured 25us/iter slower.
        xb = xbp.tile([P, Wb, NJ], bf16, name=f"xb{rc}", tag="xb")

        # fused row-DCT + transpose: pb[w, n] = sum_r xb[r, w, j] * dbm2[r, n]
        uts = []
        for wc in range(NWC):
            ws = slice(wc * P, (wc + 1) * P)
            ve_copy(nv[0], xb[:, ws, :], xin_h[:, ws, 2:BLOCK, 1])
            nv[0] += 1
            ut = utp.tile([P, NJ, NL], bf16, name=f"ut{rc}_{wc}",
                          tag=f"ut{wc}")
            # padded 2-bank psum tile: region (h2, ji) at h2*2048 + ji*512 B,
            # so no individual matmul output crosses a PSUM bank boundary
            pb = psB.tile([P, 2, 4, NL], f32, name=f"pb{rc}_{wc}", tag="pb")
            for h2 in range(2):
                for ji in range(3):
                    nc.tensor.matmul(pb[:, h2, ji, :],
                                     xb[:, ws, 3 * h2 + ji],
                                     dbm2, start=True, stop=True)
            ve_copy(nv[0], ut.rearrange("p (h j) n -> p h j n", h=2),
                    pb[:, :, 0:3, :])
            nv[0] += 1
            uts.append(ut)
        return uts

    def back(rc, uts):
        # column DCT: DwT chunk stationary (all 128 PE partitions = k used),
        # accumulate over the 4 w-chunks; one 288-wide chain per (kc, half)
        oc = ocp.tile([P, NKC, NJ, NL], bf16, name=f"oc{rc}", tag="oc")
        for kc in range(NKC):
            for h2 in range(2):
                pc = psC.tile([P, 3, NL], f32, name=f"pc{rc}_{kc}_{h2}",
                              tag="pc")
                for wc in range(NWC):
                    nc.tensor.matmul(pc, dwts[wc][:, kc * P:(kc + 1) * P],
                                     uts[wc][:, 3 * h2:3 * h2 + 3, :],
                                     start=(wc == 0), stop=(wc == NWC - 1))
                ve_copy(nv[0], oc[:, kc, 3 * h2:3 * h2 + 3, :], pc)
                nv[0] += 1
        # single compact out-DMA per rc; issued on Act (SP carries in-DMAs,
        # sharing an in-order issuing engine would head-of-line block them)
        if MODE not in ("noio", "noout"):
            nc.scalar.dma_start(out_dram[rc], oc)

    if SKEW:
        # software-pipelined emission: psC/out of rc-1 are emitted AFTER the
        # psB block of rc, so the in-order PE queue always has ready work at
        # rc boundaries (otherwise psC(rc) heads the queue waiting on ut
        # copies while psB(rc+1) inputs are already in SBUF behind it)
        pend = None
        for rc in range(NRC):
            uts = front(rc)
            if pend is not None:
                back(*pend)
            pend = (rc, uts)
        back(*pend)
    else:
        for rc in range(NRC):
            back(rc, front(rc))


_cached = {}


def _get_nc():
    if "nc" not in _cached:
        _cached["nc"] = build_nc()
    return _cached["nc"]


def _unshard(results):
    """Scatter per-core compact bf16 outputs into the full fp32 [1, H, W]."""
    full = np.zeros((N_CORES, NRC, 16, BLOCK, Wb, BLOCK), dtype=np.float32)
    for c in range(N_CORES):
        s = results[c]["out"].astype(np.float32)
        s = s.reshape(NRC, P, NKC, NJ, 16, NJ)       # [rc, k, kc, j', g, m']
        full[c, :, :, 2:, :, 2:] = (
            s.transpose(0, 4, 5, 2, 1, 3).reshape(NRC, 16, NJ, Wb, NJ))
    return full.reshape(1, H, W)


def run_sharded(x, trace=False, **kw):
    """x: [1, 4096, 4096] float32 full input. Returns (out, BassKernelResults)."""
    from concourse.bass_utils import run_bass_kernel_spmd

    nc = _get_nc()
    x = np.asarray(x, dtype=np.float32)
    assert x.shape == (1, H, W)
    consts = make_consts()
    in_maps = []
    for i in range(N_CORES):
        m = {"x": np.ascontiguousarray(x[0, i * R:(i + 1) * R, :])}
        m.update(consts)
        in_maps.append(m)
    try:
        res = run_bass_kernel_spmd(nc, in_maps, core_ids=list(range(N_CORES)),
                                   trace=trace, **kw)
    except Exception:
        # transient NRT_EXEC_UNIT_UNRECOVERABLE device faults were observed
        # on this fleet; one retry is usually enough
        res = run_bass_kernel_spmd(nc, in_maps, core_ids=list(range(N_CORES)),
                                   trace=trace, **kw)
    return _unshard(res.results), res


def kernel(x):
    out, _ = run_sharded(x, trace=False)
    return out


if __name__ == "__main__":
    rng = np.random.default_rng(0)
    x = rng.random((1, H, W), dtype=np.float32)
    out, res = run_sharded(x)
    print("out shape", out.shape, "exec_time_ns", res.exec_time_ns)
